# revision 4
# baseline (speedup 1.0000x reference)
"""GQA kernel for trn2, 8 NeuronCores.

Problem: B=1, S=2048, D=128, H=32, KVH=8, REP=4, rope(theta=1e4) on k AND v,
softmax(q@k^T/sqrt(128)) @ v, out @ Wo + bo.  The reference replicates torch
.view() semantics: (B,S,H*D) -> (B,H,S,D) is a FLAT reinterpretation, so
q-head h is rows [h*64,(h+1)*64) of the projection output reinterpreted as
(2048,128), and kv-head g is rows [g*256,(g+1)*256) of the k/v projections.

Sharding: core c owns kv-head g=c and q-heads {c, c+8, c+16, c+24}.
Device keeps everything in matmul-natural "storage order": q-position
j_q = b*64+a  <-> actual s' = 32a+b, kv-position j_k = b*256+a <-> t = 8a+b.
RoPE tables are host-permuted into storage order; host un-permutes rows of
the final output and sums partials over cores (Wo is a per-head row-block
contraction, so per-core partials add).

Dataflow per head: scores^T[jk,jq] = (KTr2 slice).T @ QT slice  (f32r),
exp via ScalarE psum->sbuf (bf16 probs), AV = V_r2[jk].T @ probs accumulated
in psum over jk tiles, denominators via all-ones lhsT matmul on the same
probs, normalize on DVE, per-head Wo matmuls accumulate output tiles.
"""

import sys

sys.path.insert(0, "/opt/trn_rl_repo")

import numpy as np
import ml_dtypes

import concourse.bass as bass
import concourse.mybir as mybir
import concourse.tile as tile
from concourse import bacc
from concourse.bass_utils import run_bass_kernel_spmd

F32 = mybir.dt.float32
F32R = mybir.dt.float32r
BF16 = mybir.dt.bfloat16

B, S, D = 1, 2048, 128
H, KVH, REP = 32, 8, 4
NCORES = 8
SCALE = 1.0 / np.sqrt(128.0)
ROPE_THETA = 10000.0

# storage-order <-> position permutations
_j = np.arange(S)
PERM_Q = 32 * (_j % 64) + _j // 64          # s' = PERM_Q[j_q]
PERM_K = 8 * (_j % 256) + _j // 256         # t  = PERM_K[j_k]

_nc_cache = {}


def _rope_tables():
    inv_freq = 1.0 / (ROPE_THETA ** (np.arange(0, D, 2, dtype=np.float64) / D))
    ang = np.arange(S, dtype=np.float64)[:, None] * inv_freq  # (S, 64)
    cos = np.cos(ang)  # (S, 64), same for d and d+64
    sin = np.sin(ang)

    # K/V-transposed layout [d, j]: value at (d, j) uses t = PERM_K[j]
    cosK = np.empty((D, S), np.float32)
    sinKe = np.empty((D, S), np.float32)
    t = PERM_K
    cosK[:64, :] = cos[t, :].T
    cosK[64:, :] = cos[t, :].T
    sinKe[:64, :] = -sin[t, :].T   # rot[d<64] = -x[d+64]
    sinKe[64:, :] = sin[t, :].T    # rot[d>=64] = +x[d-64]

    # V row layout [p, m*128+d]: row j = m*128+p, t = PERM_K[j]
    cosVr = np.empty((128, S), np.float32)
    sinVe = np.empty((128, S), np.float32)
    for m in range(16):
        tj = PERM_K[m * 128 + np.arange(128)]
        c = cos[tj, :]  # (128, 64)
        s_ = sin[tj, :]
        cosVr[:, m * 128:m * 128 + 64] = c
        cosVr[:, m * 128 + 64:m * 128 + 128] = c
        sinVe[:, m * 128:m * 128 + 64] = -s_
        sinVe[:, m * 128 + 64:m * 128 + 128] = s_
    return cosK, sinKe, cosVr, sinVe


def _build_nc():
    import os
    abl = set(os.environ.get("KABL", "").split(","))
    nc = bacc.Bacc(None)
    dp = nc.declare_dram_parameter
    qT = dp("qT", [128, 256], F32R, isOutput=False)
    kT = dp("kT", [128, 256], F32R, isOutput=False)
    vT = dp("vT", [128, 256], F32R, isOutput=False)
    wq = dp("wq", [128, H * D], F32R, isOutput=False)
    wk = dp("wk", [128, KVH * D], F32R, isOutput=False)
    wv = dp("wv", [128, KVH * D], F32R, isOutput=False)
    wo = dp("wo", [128, 4 * 128], F32R, isOutput=False)
    bq = dp("bq", [128, 32], F32, isOutput=False)
    bk = dp("bk", [128, 8], F32, isOutput=False)
    bv = dp("bv", [1, KVH * D], F32R, isOutput=False)
    cosK = dp("cosK", [128, S], F32, isOutput=False)
    sinK = dp("sinK", [128, S], F32, isOutput=False)
    cosV = dp("cosV", [128, S], F32, isOutput=False)
    sinV = dp("sinV", [128, S], F32, isOutput=False)
    onesr = dp("onesr", [1, 128], F32R, isOutput=False)
    onesf = dp("onesf", [128, 128], F32R, isOutput=False)
    out = dp("out", [128, S], F32, isOutput=True)

    ADD = mybir.AluOpType.add
    MUL = mybir.AluOpType.mult
    EXP = mybir.ActivationFunctionType.Exp

    with tile.TileContext(nc) as tc:
        with tc.tile_pool(name="cst", bufs=1) as cst, \
             tc.tile_pool(name="big", bufs=1) as big, \
             tc.tile_pool(name="pb", bufs=4) as pb, \
             tc.tile_pool(name="rc", bufs=2) as rc, \
             tc.tile_pool(name="psA", bufs=1, space="PSUM") as psA, \
             tc.tile_pool(name="psB", bufs=1, space="PSUM") as psB, \
             tc.tile_pool(name="psC", bufs=2, space="PSUM") as psC:
            # ---- load constants/inputs ----
            wq_sb = cst.tile([128, H * D], F32R, tag="wq")
            wk_sb = cst.tile([128, KVH * D], F32R, tag="wk")
            wv_sb = cst.tile([128, KVH * D], F32R, tag="wv")
            wo_sb = cst.tile([128, 512], F32R, tag="wo")
            qT_sb = cst.tile([128, 256], F32R, tag="qT")
            kT_sb = cst.tile([128, 256], F32R, tag="kT")
            vT_sb = cst.tile([128, 256], F32R, tag="vT")
            bq_sb = cst.tile([128, 32], F32, tag="bq")
            bk_sb = cst.tile([128, 8], F32, tag="bk")
            bv_sb = cst.tile([1, KVH * D], F32R, tag="bv")
            cosK_sb = cst.tile([128, S], F32, tag="cosK")
            sinK_sb = cst.tile([128, S], F32, tag="sinK")
            cosV_sb = cst.tile([128, S], F32, tag="cosV")
            sinV_sb = cst.tile([128, S], F32, tag="sinV")
            onesr_sb = cst.tile([1, 128], F32R, tag="onesr")
            onesf_sb = cst.tile([128, 128], F32R, tag="onesf")
            for t_, d_ in ((wq_sb, wq), (qT_sb, qT), (kT_sb, kT), (vT_sb, vT),
                           (wk_sb, wk), (wv_sb, wv), (wo_sb, wo), (bq_sb, bq),
                           (bk_sb, bk), (bv_sb, bv), (cosK_sb, cosK),
                           (sinK_sb, sinK), (cosV_sb, cosV), (sinV_sb, sinV),
                           (onesr_sb, onesr), (onesf_sb, onesf)):
                nc.sync.dma_start(out=t_[:], in_=d_[:])

            # ---- Q projection: QT_all[d, h, b, a] ----
            QT = big.tile([128, 4 * S], F32R, tag="QT")
            qt4 = QT[:].rearrange("p (h b a) -> p h b a", h=4, b=32)
            for b in range(32):
                pq = psC.tile([128, 256], F32, tag="sc")
                nc.tensor.matmul(pq[:], wq_sb[:, b * 128:(b + 1) * 128],
                                 qT_sb[:], start=True, stop=True)
                nc.vector.tensor_scalar(
                    qt4[:, :, b, :], pq[:].rearrange("p (h a) -> p h a", h=4),
                    bq_sb[:, b:b + 1], None, ADD)

            # ---- K projection + rope: KTr2[d, jk] ----
            KT = big.tile([128, S], F32R, tag="KT")
            for b in range(8):
                pk = psC.tile([128, 256], F32, tag="sc")
                nc.tensor.matmul(pk[:], wk_sb[:, b * 128:(b + 1) * 128],
                                 kT_sb[:], start=True, stop=True)
                nc.vector.tensor_scalar(KT[:, b * 256:(b + 1) * 256], pk[:],
                                        bk_sb[:, b:b + 1], None, ADD)
            tmpK = big.tile([128, S], F32, tag="tmpK")
            nc.vector.tensor_copy(tmpK[0:64, :], KT[64:128, :])
            nc.vector.tensor_copy(tmpK[64:128, :], KT[0:64, :])
            nc.vector.tensor_tensor(tmpK[:], tmpK[:], sinK_sb[:], MUL)
            nc.vector.tensor_tensor(KT[:], KT[:], cosK_sb[:], MUL)
            nc.vector.tensor_tensor(KT[:], KT[:], tmpK[:], ADD)

            # ---- V projection + rope in row layout: V_r2[p, m, d] ----
            VR = big.tile([128, S], F32R, tag="VR")
            vr4 = VR[:].rearrange("p (b two d) -> p b two d", b=8, two=2)
            for bg in range(2):
                for ah in range(2):
                    pv = psC.tile([128, 512], F32, tag="sc")
                    nc.tensor.matmul(pv[:], onesr_sb[:],
                                     bv_sb[:, bg * 512:(bg + 1) * 512],
                                     start=True, stop=False)
                    nc.tensor.matmul(pv[:], vT_sb[:, ah * 128:(ah + 1) * 128],
                                     wv_sb[:, bg * 512:(bg + 1) * 512],
                                     start=False, stop=True,
                                     skip_group_check=True)
                    nc.vector.tensor_copy(
                        vr4[:, 4 * bg:4 * bg + 4, ah, :],
                        pv[:].rearrange("p (b d) -> p b d", b=4))
            tmpV = big.tile([128, S], F32, tag="tmpV")
            vr3 = VR[:].rearrange("p (m h d) -> p m h d", m=16, h=2)
            tv3 = tmpV[:].rearrange("p (m h d) -> p m h d", m=16, h=2)
            sv3 = sinV_sb[:].rearrange("p (m h d) -> p m h d", m=16, h=2)
            nc.vector.tensor_tensor(tv3[:, :, 0, :], vr3[:, :, 1, :],
                                    sv3[:, :, 0, :], MUL)
            nc.vector.tensor_tensor(tv3[:, :, 1, :], vr3[:, :, 0, :],
                                    sv3[:, :, 1, :], MUL)
            nc.vector.tensor_tensor(VR[:], VR[:], cosV_sb[:], MUL)
            nc.vector.tensor_tensor(VR[:], VR[:], tmpV[:], ADD)
            vr2t = VR[:].rearrange("p (m d) -> p m d", m=16)

            # ---- attention per head ----
            OHT = big.tile([128, 4 * S], F32R, tag="OHT")
            for h in range(4):
                for half in range(2):
                    av = psA.tile([128, 1024], F32, tag="av")
                    dn = psB.tile([128, 1024], F32, tag="dn")
                    for jk in range(16):
                        sc = psC.tile([128, 1024], F32, tag="sc")
                        for c in range(2):
                            nc.tensor.matmul(
                                sc[:, c * 512:(c + 1) * 512],
                                KT[:, jk * 128:(jk + 1) * 128],
                                QT[:, h * S + half * 1024 + c * 512:
                                   h * S + half * 1024 + (c + 1) * 512],
                                start=True, stop=True)
                        pr = pb.tile([128, 1024], F32R, tag="pr")
                        if "noexp" in abl:
                            nc.vector.tensor_copy(pr[:], sc[:])
                        else:
                            nc.scalar.activation(pr[:], sc[:], EXP, scale=SCALE)
                        for c in range(2):
                            cs = slice(c * 512, (c + 1) * 512)
                            if "noav" not in abl:
                                nc.tensor.matmul(av[:, cs], vr2t[:, jk, :],
                                                 pr[:, cs],
                                                 start=(jk == 0),
                                                 stop=(jk == 15),
                                                 skip_group_check=True)
                            if "nodn" not in abl:
                                nc.tensor.matmul(dn[:, cs], onesf_sb[:],
                                                 pr[:, cs],
                                                 start=(jk == 0),
                                                 stop=(jk == 15),
                                                 skip_group_check=True)
                    rcp = rc.tile([128, 1024], F32, tag="rcp")
                    nc.vector.reciprocal(rcp[:], av[:] if "nodn" in abl else dn[:])
                    nc.vector.tensor_tensor(
                        OHT[:, h * S + half * 1024:h * S + (half + 1) * 1024],
                        dn[:] if "noav" in abl else av[:], rcp[:], MUL)

            # ---- output projection (accumulate 4 heads) ----
            out_sb = big.tile([128, S], F32, tag="osb")
            for jc in range(4):
                po = psC.tile([128, 512], F32, tag="sc")
                for h in range(4):
                    nc.tensor.matmul(po[:],
                                     wo_sb[:, h * 128:(h + 1) * 128],
                                     OHT[:, h * S + jc * 512:
                                         h * S + (jc + 1) * 512],
                                     start=(h == 0), stop=(h == 3),
                                     skip_group_check=True)
                nc.vector.tensor_copy(out_sb[:, jc * 512:(jc + 1) * 512],
                                      po[:])
            nc.sync.dma_start(out=out[:], in_=out_sb[:])

    nc.compile()
    return nc


def _get_nc():
    if "nc" not in _nc_cache:
        _nc_cache["nc"] = _build_nc()
    return _nc_cache["nc"]


def make_in_maps(query, keys, values, Wq, bq, Wk, bk, Wv, bv, Wo, bo):
    cosK, sinKe, cosVr, sinVe = _rope_tables()
    q2 = np.asarray(query, np.float32).reshape(S, D)
    k2 = np.asarray(keys, np.float32).reshape(S, D)
    v2 = np.asarray(values, np.float32).reshape(S, D)
    Wq_ = np.ascontiguousarray(np.asarray(Wq, np.float32))
    Wk_ = np.ascontiguousarray(np.asarray(Wk, np.float32))
    Wv_ = np.ascontiguousarray(np.asarray(Wv, np.float32))
    Wo_ = np.asarray(Wo, np.float32)
    bq_ = np.asarray(bq, np.float32).reshape(32, 128).T.copy()   # [d, b]
    bk_ = np.asarray(bk, np.float32).reshape(8, 128).T.copy()
    bv_ = np.asarray(bv, np.float32).reshape(1, KVH * D).copy()
    ones_r = np.ones((1, 128), np.float32)
    ones_f = np.ones((128, 128), np.float32)

    in_maps = []
    for c in range(NCORES):
        heads = [c + 8 * r for r in range(REP)]
        qrows = np.concatenate([q2[hh * 64:(hh + 1) * 64] for hh in heads])
        woc = np.concatenate([Wo_[hh * 128:(hh + 1) * 128] for hh in heads],
                             axis=1)  # [128, 4*128]
        in_maps.append({
            "qT": np.ascontiguousarray(qrows.T),
            "kT": np.ascontiguousarray(k2[c * 256:(c + 1) * 256].T),
            "vT": np.ascontiguousarray(v2[c * 256:(c + 1) * 256].T),
            "wq": Wq_, "wk": Wk_, "wv": Wv_,
            "wo": np.ascontiguousarray(woc),
            "bq": bq_, "bk": bk_, "bv": bv_,
            "cosK": cosK, "sinK": sinKe, "cosV": cosVr, "sinV": sinVe,
            "onesr": ones_r, "onesf": ones_f,
        })
    return in_maps


def kernel(query, keys, values, Wq, bq, Wk, bk, Wv, bv, Wo, bo):
    nc = _get_nc()
    in_maps = make_in_maps(query, keys, values, Wq, bq, Wk, bk, Wv, bv, Wo, bo)
    res = run_bass_kernel_spmd(nc, in_maps, list(range(NCORES)))
    return postprocess(res.results, bo)


def postprocess(results, bo):
    acc = np.zeros((S, D), np.float64)
    for c in range(NCORES):
        o = np.asarray(results[c]["out"], np.float32)  # [dout=128, jq=2048]
        acc += o.T
    final = np.empty((S, D), np.float32)
    final[PERM_Q] = acc.astype(np.float32)
    final += np.asarray(bo, np.float32)
    return final.reshape(B, S, D)



# revision 5
# speedup vs baseline: 1.1761x; 1.1761x over previous
"""GQA kernel for trn2, 8 NeuronCores.

Problem: B=1, S=2048, D=128, H=32, KVH=8, REP=4, rope(theta=1e4) on k AND v,
softmax(q@k^T/sqrt(128)) @ v, out @ Wo + bo.  The reference replicates torch
.view() semantics: (B,S,H*D) -> (B,H,S,D) is a FLAT reinterpretation, so
q-head h is rows [h*64,(h+1)*64) of the projection output reinterpreted as
(2048,128), and kv-head g is rows [g*256,(g+1)*256) of the k/v projections.

Sharding: core c owns kv-head g=c and q-heads {c, c+8, c+16, c+24}.
Device keeps everything in matmul-natural "storage order": q-position
j_q = b*64+a  <-> actual s' = 32a+b, kv-position j_k = b*256+a <-> t = 8a+b.
RoPE tables are host-permuted into storage order; host un-permutes rows of
the final output and sums partials over cores (Wo is a per-head row-block
contraction, so per-core partials add).

Dataflow per head: scores^T[jk,jq] = (KTr2 slice).T @ QT slice  (f32r),
exp via ScalarE psum->sbuf (bf16 probs), AV = V_r2[jk].T @ probs accumulated
in psum over jk tiles, denominators via all-ones lhsT matmul on the same
probs, normalize on DVE, per-head Wo matmuls accumulate output tiles.
"""

import sys

sys.path.insert(0, "/opt/trn_rl_repo")

import numpy as np
import ml_dtypes

import concourse.bass as bass
import concourse.mybir as mybir
import concourse.tile as tile
from concourse import bacc
from concourse.bass_utils import run_bass_kernel_spmd

F32 = mybir.dt.float32
F32R = mybir.dt.float32r
BF16 = mybir.dt.bfloat16

B, S, D = 1, 2048, 128
H, KVH, REP = 32, 8, 4
NCORES = 8
SCALE = 1.0 / np.sqrt(128.0)
ROPE_THETA = 10000.0

# storage-order <-> position permutations
_j = np.arange(S)
PERM_Q = 32 * (_j % 64) + _j // 64          # s' = PERM_Q[j_q]
PERM_K = 8 * (_j % 256) + _j // 256         # t  = PERM_K[j_k]

_nc_cache = {}


def _rope_tables():
    inv_freq = 1.0 / (ROPE_THETA ** (np.arange(0, D, 2, dtype=np.float64) / D))
    ang = np.arange(S, dtype=np.float64)[:, None] * inv_freq  # (S, 64)
    cos = np.cos(ang)  # (S, 64), same for d and d+64
    sin = np.sin(ang)

    # K/V-transposed layout [d, j]: value at (d, j) uses t = PERM_K[j]
    cosK = np.empty((D, S), np.float32)
    sinKe = np.empty((D, S), np.float32)
    t = PERM_K
    cosK[:64, :] = cos[t, :].T
    cosK[64:, :] = cos[t, :].T
    sinKe[:64, :] = -sin[t, :].T   # rot[d<64] = -x[d+64]
    sinKe[64:, :] = sin[t, :].T    # rot[d>=64] = +x[d-64]

    # V row layout [p, m*128+d]: row j = m*128+p, t = PERM_K[j]
    cosVr = np.empty((128, S), np.float32)
    sinVe = np.empty((128, S), np.float32)
    for m in range(16):
        tj = PERM_K[m * 128 + np.arange(128)]
        c = cos[tj, :]  # (128, 64)
        s_ = sin[tj, :]
        cosVr[:, m * 128:m * 128 + 64] = c
        cosVr[:, m * 128 + 64:m * 128 + 128] = c
        sinVe[:, m * 128:m * 128 + 64] = -s_
        sinVe[:, m * 128 + 64:m * 128 + 128] = s_
    return cosK, sinKe, cosVr, sinVe


def _build_nc():
    import os
    abl = set(os.environ.get("KABL", "").split(","))
    nc = bacc.Bacc(None)
    dp = nc.declare_dram_parameter
    qT = dp("qT", [128, 256], F32R, isOutput=False)
    kT = dp("kT", [128, 256], F32R, isOutput=False)
    vT = dp("vT", [128, 256], F32R, isOutput=False)
    wq = dp("wq", [128, H * D], F32R, isOutput=False)
    wk = dp("wk", [128, KVH * D], F32R, isOutput=False)
    wv = dp("wv", [128, KVH * D], F32R, isOutput=False)
    wo = dp("wo", [128, 4 * 128], F32R, isOutput=False)
    bq = dp("bq", [128, 32], F32, isOutput=False)
    bk = dp("bk", [128, 8], F32, isOutput=False)
    bv = dp("bv", [1, KVH * D], F32R, isOutput=False)
    cosK = dp("cosK", [128, S], F32, isOutput=False)
    sinK = dp("sinK", [128, S], F32, isOutput=False)
    cosV = dp("cosV", [128, S], F32, isOutput=False)
    sinV = dp("sinV", [128, S], F32, isOutput=False)
    onesr = dp("onesr", [1, 128], F32R, isOutput=False)
    onesf = dp("onesf", [128, 128], F32R, isOutput=False)
    out = dp("out", [128, S], F32, isOutput=True)

    ADD = mybir.AluOpType.add
    MUL = mybir.AluOpType.mult
    EXP = mybir.ActivationFunctionType.Exp

    with tile.TileContext(nc) as tc:
        with tc.tile_pool(name="cst", bufs=1) as cst, \
             tc.tile_pool(name="big", bufs=1) as big, \
             tc.tile_pool(name="pb", bufs=4) as pb, \
             tc.tile_pool(name="rc", bufs=2) as rc, \
             tc.tile_pool(name="psA", bufs=1, space="PSUM") as psA, \
             tc.tile_pool(name="psB", bufs=1, space="PSUM") as psB, \
             tc.tile_pool(name="psC", bufs=2, space="PSUM") as psC:
            # ---- load constants/inputs ----
            wq_sb = cst.tile([128, H * D], F32R, tag="wq")
            wk_sb = cst.tile([128, KVH * D], F32R, tag="wk")
            wv_sb = cst.tile([128, KVH * D], F32R, tag="wv")
            wo_sb = cst.tile([128, 512], F32R, tag="wo")
            qT_sb = cst.tile([128, 256], F32R, tag="qT")
            kT_sb = cst.tile([128, 256], F32R, tag="kT")
            vT_sb = cst.tile([128, 256], F32R, tag="vT")
            bq_sb = cst.tile([128, 32], F32, tag="bq")
            bk_sb = cst.tile([128, 8], F32, tag="bk")
            bv_sb = cst.tile([1, KVH * D], F32R, tag="bv")
            cosK_sb = cst.tile([128, S], F32, tag="cosK")
            sinK_sb = cst.tile([128, S], F32, tag="sinK")
            cosV_sb = cst.tile([128, S], F32, tag="cosV")
            sinV_sb = cst.tile([128, S], F32, tag="sinV")
            onesr_sb = cst.tile([1, 128], F32R, tag="onesr")
            onesf_sb = cst.tile([128, 128], F32R, tag="onesf")
            for t_, d_ in ((wq_sb, wq), (qT_sb, qT), (kT_sb, kT), (vT_sb, vT),
                           (wk_sb, wk), (wv_sb, wv), (wo_sb, wo), (bq_sb, bq),
                           (bk_sb, bk), (bv_sb, bv), (cosK_sb, cosK),
                           (sinK_sb, sinK), (cosV_sb, cosV), (sinV_sb, sinV),
                           (onesr_sb, onesr), (onesf_sb, onesf)):
                nc.sync.dma_start(out=t_[:], in_=d_[:])

            # ---- Q projection: QT_all[d, h, b, a] ----
            QT = big.tile([128, 4 * S], F32R, tag="QT")
            qt4 = QT[:].rearrange("p (h b a) -> p h b a", h=4, b=32)
            for b in range(32):
                pq = psC.tile([128, 256], F32, tag="sc")
                nc.tensor.matmul(pq[:], wq_sb[:, b * 128:(b + 1) * 128],
                                 qT_sb[:], start=True, stop=True)
                nc.vector.tensor_scalar(
                    qt4[:, :, b, :], pq[:].rearrange("p (h a) -> p h a", h=4),
                    bq_sb[:, b:b + 1], None, ADD)

            # ---- K projection + rope: KTr2[d, jk] ----
            KT = big.tile([128, S], F32R, tag="KT")
            for b in range(8):
                pk = psC.tile([128, 256], F32, tag="sc")
                nc.tensor.matmul(pk[:], wk_sb[:, b * 128:(b + 1) * 128],
                                 kT_sb[:], start=True, stop=True)
                nc.vector.tensor_scalar(KT[:, b * 256:(b + 1) * 256], pk[:],
                                        bk_sb[:, b:b + 1], None, ADD)
            tmpK = big.tile([128, S], F32, tag="tmpK")
            nc.vector.tensor_copy(tmpK[0:64, :], KT[64:128, :])
            nc.vector.tensor_copy(tmpK[64:128, :], KT[0:64, :])
            nc.vector.tensor_tensor(tmpK[:], tmpK[:], sinK_sb[:], MUL)
            nc.vector.tensor_tensor(KT[:], KT[:], cosK_sb[:], MUL)
            nc.vector.tensor_tensor(KT[:], KT[:], tmpK[:], ADD)

            # ---- V projection + rope in row layout: V_r2[p, m, d] ----
            VR = big.tile([128, S], F32R, tag="VR")
            vr4 = VR[:].rearrange("p (b two d) -> p b two d", b=8, two=2)
            for bg in range(2):
                for ah in range(2):
                    pv = psC.tile([128, 512], F32, tag="sc")
                    nc.tensor.matmul(pv[:], onesr_sb[:],
                                     bv_sb[:, bg * 512:(bg + 1) * 512],
                                     start=True, stop=False)
                    nc.tensor.matmul(pv[:], vT_sb[:, ah * 128:(ah + 1) * 128],
                                     wv_sb[:, bg * 512:(bg + 1) * 512],
                                     start=False, stop=True,
                                     skip_group_check=True)
                    nc.vector.tensor_copy(
                        vr4[:, 4 * bg:4 * bg + 4, ah, :],
                        pv[:].rearrange("p (b d) -> p b d", b=4))
            tmpV = big.tile([128, S], F32, tag="tmpV")
            vr3 = VR[:].rearrange("p (m h d) -> p m h d", m=16, h=2)
            tv3 = tmpV[:].rearrange("p (m h d) -> p m h d", m=16, h=2)
            sv3 = sinV_sb[:].rearrange("p (m h d) -> p m h d", m=16, h=2)
            nc.vector.tensor_tensor(tv3[:, :, 0, :], vr3[:, :, 1, :],
                                    sv3[:, :, 0, :], MUL)
            nc.vector.tensor_tensor(tv3[:, :, 1, :], vr3[:, :, 0, :],
                                    sv3[:, :, 1, :], MUL)
            nc.vector.tensor_tensor(VR[:], VR[:], cosV_sb[:], MUL)
            nc.vector.tensor_tensor(VR[:], VR[:], tmpV[:], ADD)
            vr2t = VR[:].rearrange("p (m d) -> p m d", m=16)

            # ---- attention per head (software-pipelined over jk) ----
            # Per (h, half) group: scores for tile jk are emitted two
            # iterations ahead of the av/dn matmuls that consume exp(jk),
            # so the PE never waits on the ScalarE exp.  PSUM budget:
            # av(2) + dn(2) + 2 in-flight sc tiles (4) = 8 banks.
            OHT = big.tile([128, 4 * S], F32R, tag="OHT")
            for h in range(4):
                for half in range(2):
                    base = h * S + half * 1024
                    av = psA.tile([128, 1024], F32, tag="av")
                    dn = psB.tile([128, 1024], F32, tag="dn")
                    prs = {}

                    def emit_sc(jk):
                        sc = psC.tile([128, 1024], F32, tag="sc")
                        for c in range(2):
                            nc.tensor.matmul(
                                sc[:, c * 512:(c + 1) * 512],
                                KT[:, jk * 128:(jk + 1) * 128],
                                QT[:, base + c * 512:base + (c + 1) * 512],
                                start=True, stop=True)
                        pr = pb.tile([128, 1024], F32R, tag="pr")
                        nc.scalar.activation(pr[:], sc[:], EXP, scale=SCALE)
                        prs[jk] = pr

                    def emit_avd(jk):
                        pr = prs.pop(jk)
                        for c in range(2):
                            cs = slice(c * 512, (c + 1) * 512)
                            nc.tensor.matmul(av[:, cs], vr2t[:, jk, :],
                                             pr[:, cs],
                                             start=(jk == 0),
                                             stop=(jk == 15),
                                             skip_group_check=True)
                            nc.tensor.matmul(dn[:, cs], onesf_sb[:],
                                             pr[:, cs],
                                             start=(jk == 0),
                                             stop=(jk == 15),
                                             skip_group_check=True)

                    emit_sc(0)
                    emit_sc(1)
                    for jk in range(16):
                        emit_avd(jk)
                        if jk + 2 < 16:
                            emit_sc(jk + 2)
                    rcp = rc.tile([128, 1024], F32, tag="rcp")
                    nc.vector.reciprocal_approx_fast(rcp[:], dn[:])
                    nc.vector.tensor_tensor(OHT[:, base:base + 1024],
                                            av[:], rcp[:], MUL)

            # ---- output projection (accumulate 4 heads) ----
            out_sb = big.tile([128, S], F32, tag="osb")
            for jc in range(4):
                po = psC.tile([128, 512], F32, tag="sc")
                for h in range(4):
                    nc.tensor.matmul(po[:],
                                     wo_sb[:, h * 128:(h + 1) * 128],
                                     OHT[:, h * S + jc * 512:
                                         h * S + (jc + 1) * 512],
                                     start=(h == 0), stop=(h == 3),
                                     skip_group_check=True)
                nc.vector.tensor_copy(out_sb[:, jc * 512:(jc + 1) * 512],
                                      po[:])
            nc.sync.dma_start(out=out[:], in_=out_sb[:])

    nc.compile()
    return nc


def _get_nc():
    if "nc" not in _nc_cache:
        _nc_cache["nc"] = _build_nc()
    return _nc_cache["nc"]


def make_in_maps(query, keys, values, Wq, bq, Wk, bk, Wv, bv, Wo, bo):
    cosK, sinKe, cosVr, sinVe = _rope_tables()
    q2 = np.asarray(query, np.float32).reshape(S, D)
    k2 = np.asarray(keys, np.float32).reshape(S, D)
    v2 = np.asarray(values, np.float32).reshape(S, D)
    Wq_ = np.ascontiguousarray(np.asarray(Wq, np.float32))
    Wk_ = np.ascontiguousarray(np.asarray(Wk, np.float32))
    Wv_ = np.ascontiguousarray(np.asarray(Wv, np.float32))
    Wo_ = np.asarray(Wo, np.float32)
    bq_ = np.asarray(bq, np.float32).reshape(32, 128).T.copy()   # [d, b]
    bk_ = np.asarray(bk, np.float32).reshape(8, 128).T.copy()
    bv_ = np.asarray(bv, np.float32).reshape(1, KVH * D).copy()
    ones_r = np.ones((1, 128), np.float32)
    ones_f = np.ones((128, 128), np.float32)

    in_maps = []
    for c in range(NCORES):
        heads = [c + 8 * r for r in range(REP)]
        qrows = np.concatenate([q2[hh * 64:(hh + 1) * 64] for hh in heads])
        woc = np.concatenate([Wo_[hh * 128:(hh + 1) * 128] for hh in heads],
                             axis=1)  # [128, 4*128]
        in_maps.append({
            "qT": np.ascontiguousarray(qrows.T),
            "kT": np.ascontiguousarray(k2[c * 256:(c + 1) * 256].T),
            "vT": np.ascontiguousarray(v2[c * 256:(c + 1) * 256].T),
            "wq": Wq_, "wk": Wk_, "wv": Wv_,
            "wo": np.ascontiguousarray(woc),
            "bq": bq_, "bk": bk_, "bv": bv_,
            "cosK": cosK, "sinK": sinKe, "cosV": cosVr, "sinV": sinVe,
            "onesr": ones_r, "onesf": ones_f,
        })
    return in_maps


def kernel(query, keys, values, Wq, bq, Wk, bk, Wv, bv, Wo, bo):
    nc = _get_nc()
    in_maps = make_in_maps(query, keys, values, Wq, bq, Wk, bk, Wv, bv, Wo, bo)
    res = run_bass_kernel_spmd(nc, in_maps, list(range(NCORES)))
    return postprocess(res.results, bo)


def postprocess(results, bo):
    acc = np.zeros((S, D), np.float64)
    for c in range(NCORES):
        o = np.asarray(results[c]["out"], np.float32)  # [dout=128, jq=2048]
        acc += o.T
    final = np.empty((S, D), np.float32)
    final[PERM_Q] = acc.astype(np.float32)
    final += np.asarray(bo, np.float32)
    return final.reshape(B, S, D)



# revision 6
# speedup vs baseline: 1.5677x; 1.3330x over previous
"""GQA kernel for trn2, 8 NeuronCores.

Problem: B=1, S=2048, D=128, H=32, KVH=8, REP=4, rope(theta=1e4) on k AND v,
softmax(q@k^T/sqrt(128)) @ v, out @ Wo + bo.  The reference replicates torch
.view() semantics: (B,S,H*D) -> (B,H,S,D) is a FLAT reinterpretation, so
q-head h is rows [h*64,(h+1)*64) of the projection output reinterpreted as
(2048,128), and kv-head g is rows [g*256,(g+1)*256) of the k/v projections.

Sharding: core c owns kv-head g=c and q-heads {c, c+8, c+16, c+24}.
Device keeps everything in matmul-natural "storage order": q-position
j_q = b*64+a  <-> actual s' = 32a+b, kv-position j_k = b*256+a <-> t = 8a+b.
RoPE tables are host-permuted into storage order; host un-permutes rows of
the final output and sums partials over cores (Wo is a per-head row-block
contraction, so per-core partials add).

Dataflow per head: scores^T[jk,jq] = (KTr2 slice).T @ QT slice (bf16),
exp via ScalarE psum->sbuf (bf16 probs), AV = V_r2[jk].T @ probs accumulated
in psum over jk tiles, denominators via all-ones lhsT matmul on the same
probs, fast-reciprocal + normalize on DVE, per-head Wo matmuls accumulate
output tiles.  The jk loop is software-pipelined: scores for tile jk+2 are
issued before the av/dn matmuls of tile jk so the PE never waits on exp.
"""

import sys

sys.path.insert(0, "/opt/trn_rl_repo")

import numpy as np
import ml_dtypes

import concourse.bass as bass
import concourse.mybir as mybir
import concourse.tile as tile
from concourse import bacc
from concourse.bass_utils import run_bass_kernel_spmd

F32 = mybir.dt.float32
F32R = mybir.dt.float32r
BF16 = mybir.dt.bfloat16

B, S, D = 1, 2048, 128
H, KVH, REP = 32, 8, 4
NCORES = 8
SCALE = 1.0 / np.sqrt(128.0)
ROPE_THETA = 10000.0

# storage-order <-> position permutations
_j = np.arange(S)
PERM_Q = 32 * (_j % 64) + _j // 64          # s' = PERM_Q[j_q]
PERM_K = 8 * (_j % 256) + _j // 256         # t  = PERM_K[j_k]

_nc_cache = {}


def _rope_tables():
    inv_freq = 1.0 / (ROPE_THETA ** (np.arange(0, D, 2, dtype=np.float64) / D))
    ang = np.arange(S, dtype=np.float64)[:, None] * inv_freq  # (S, 64)
    cos = np.cos(ang)  # (S, 64), same for d and d+64
    sin = np.sin(ang)

    # K-transposed layout [d, j]: value at (d, j) uses t = PERM_K[j]
    cosK = np.empty((D, S), np.float32)
    sinKe = np.empty((D, S), np.float32)
    t = PERM_K
    cosK[:64, :] = cos[t, :].T
    cosK[64:, :] = cos[t, :].T
    sinKe[:64, :] = -sin[t, :].T   # rot[d<64] = -x[d+64]
    sinKe[64:, :] = sin[t, :].T    # rot[d>=64] = +x[d-64]

    # V row layout [p, m*128+d]: row j = m*128+p, t = PERM_K[j]
    cosVr = np.empty((128, S), np.float32)
    sinVe = np.empty((128, S), np.float32)
    for m in range(16):
        tj = PERM_K[m * 128 + np.arange(128)]
        c = cos[tj, :]  # (128, 64)
        s_ = sin[tj, :]
        cosVr[:, m * 128:m * 128 + 64] = c
        cosVr[:, m * 128 + 64:m * 128 + 128] = c
        sinVe[:, m * 128:m * 128 + 64] = -s_
        sinVe[:, m * 128 + 64:m * 128 + 128] = s_
    return cosK, sinKe, cosVr, sinVe


def _build_nc():
    nc = bacc.Bacc(None)
    dp = nc.declare_dram_parameter
    qT = dp("qT", [128, 256], BF16, isOutput=False)
    kT = dp("kT", [128, 256], BF16, isOutput=False)
    vT = dp("vT", [128, 256], BF16, isOutput=False)
    wq = dp("wq", [128, H * D], BF16, isOutput=False)
    wk = dp("wk", [128, KVH * D], BF16, isOutput=False)
    wv = dp("wv", [128, KVH * D], BF16, isOutput=False)
    wo = dp("wo", [128, 4 * 128], BF16, isOutput=False)
    bq = dp("bq", [128, 32], F32, isOutput=False)
    bk = dp("bk", [128, 8], F32, isOutput=False)
    bv = dp("bv", [1, KVH * D], BF16, isOutput=False)
    cosK = dp("cosK", [128, S], BF16, isOutput=False)
    sinK = dp("sinK", [128, S], BF16, isOutput=False)
    cosV = dp("cosV", [128, S], BF16, isOutput=False)
    sinV = dp("sinV", [128, S], BF16, isOutput=False)
    onesr = dp("onesr", [1, 128], BF16, isOutput=False)
    onesf = dp("onesf", [128, 128], BF16, isOutput=False)
    out = dp("out", [128, S], F32, isOutput=True)

    ADD = mybir.AluOpType.add
    MUL = mybir.AluOpType.mult
    EXP = mybir.ActivationFunctionType.Exp

    with tile.TileContext(nc) as tc:
        with tc.tile_pool(name="cst", bufs=1) as cst, \
             tc.tile_pool(name="big", bufs=1) as big, \
             tc.tile_pool(name="pb", bufs=4) as pb, \
             tc.tile_pool(name="rc", bufs=2) as rc, \
             tc.tile_pool(name="psA", bufs=1, space="PSUM") as psA, \
             tc.tile_pool(name="psB", bufs=1, space="PSUM") as psB, \
             tc.tile_pool(name="psC", bufs=2, space="PSUM") as psC:
            # ---- load constants/inputs (DMA emission order = need order:
            # K-proj inputs first, then V, then Q, then Wo/ones) ----
            kT_sb = cst.tile([128, 256], BF16, tag="kT")
            wk_sb = cst.tile([128, KVH * D], BF16, tag="wk")
            bk_sb = cst.tile([128, 8], F32, tag="bk")
            cosK_sb = cst.tile([128, S], BF16, tag="cosK")
            sinK_sb = cst.tile([128, S], BF16, tag="sinK")
            vT_sb = cst.tile([128, 256], BF16, tag="vT")
            wv_sb = cst.tile([128, KVH * D], BF16, tag="wv")
            bv_sb = cst.tile([1, KVH * D], BF16, tag="bv")
            onesr_sb = cst.tile([1, 128], BF16, tag="onesr")
            cosV_sb = cst.tile([128, S], BF16, tag="cosV")
            sinV_sb = cst.tile([128, S], BF16, tag="sinV")
            qT_sb = cst.tile([128, 256], BF16, tag="qT")
            wq_sb = cst.tile([128, H * D], BF16, tag="wq")
            bq_sb = cst.tile([128, 32], F32, tag="bq")
            onesf_sb = cst.tile([128, 128], BF16, tag="onesf")
            wo_sb = cst.tile([128, 512], BF16, tag="wo")
            for t_, d_ in ((kT_sb, kT), (wk_sb, wk), (bk_sb, bk),
                           (cosK_sb, cosK), (sinK_sb, sinK),
                           (vT_sb, vT), (wv_sb, wv), (bv_sb, bv),
                           (onesr_sb, onesr), (cosV_sb, cosV),
                           (sinV_sb, sinV),
                           (qT_sb, qT), (wq_sb, wq), (bq_sb, bq),
                           (onesf_sb, onesf), (wo_sb, wo)):
                nc.sync.dma_start(out=t_[:], in_=d_[:])

            # ---- K projection + rope: KTr2[d, jk] ----
            KT = big.tile([128, S], BF16, tag="KT")
            for b in range(8):
                pk = psC.tile([128, 256], F32, tag="sc")
                nc.tensor.matmul(pk[:], wk_sb[:, b * 128:(b + 1) * 128],
                                 kT_sb[:], start=True, stop=True)
                nc.vector.tensor_scalar(KT[:, b * 256:(b + 1) * 256], pk[:],
                                        bk_sb[:, b:b + 1], None, ADD)
            tmpK = big.tile([128, S], BF16, tag="tmpK")
            nc.vector.tensor_copy(tmpK[0:64, :], KT[64:128, :])
            nc.vector.tensor_copy(tmpK[64:128, :], KT[0:64, :])
            nc.vector.tensor_tensor(tmpK[:], tmpK[:], sinK_sb[:], MUL)
            nc.vector.tensor_tensor(KT[:], KT[:], cosK_sb[:], MUL)
            nc.vector.tensor_tensor(KT[:], KT[:], tmpK[:], ADD)

            # ---- V projection + rope in row layout: V_r2[p, m, d] ----
            VR = big.tile([128, S], BF16, tag="VR")
            vr4 = VR[:].rearrange("p (b two d) -> p b two d", b=8, two=2)
            for bg in range(2):
                for ah in range(2):
                    pv = psC.tile([128, 512], F32, tag="sc")
                    nc.tensor.matmul(pv[:], onesr_sb[:],
                                     bv_sb[:, bg * 512:(bg + 1) * 512],
                                     start=True, stop=False)
                    nc.tensor.matmul(pv[:], vT_sb[:, ah * 128:(ah + 1) * 128],
                                     wv_sb[:, bg * 512:(bg + 1) * 512],
                                     start=False, stop=True,
                                     skip_group_check=True)
                    nc.vector.tensor_copy(
                        vr4[:, 4 * bg:4 * bg + 4, ah, :],
                        pv[:].rearrange("p (b d) -> p b d", b=4))
            tmpV = big.tile([128, S], BF16, tag="tmpV")
            vr3 = VR[:].rearrange("p (m h d) -> p m h d", m=16, h=2)
            tv3 = tmpV[:].rearrange("p (m h d) -> p m h d", m=16, h=2)
            sv3 = sinV_sb[:].rearrange("p (m h d) -> p m h d", m=16, h=2)
            nc.vector.tensor_tensor(tv3[:, :, 0, :], vr3[:, :, 1, :],
                                    sv3[:, :, 0, :], MUL)
            nc.vector.tensor_tensor(tv3[:, :, 1, :], vr3[:, :, 0, :],
                                    sv3[:, :, 1, :], MUL)
            nc.vector.tensor_tensor(VR[:], VR[:], cosV_sb[:], MUL)
            nc.vector.tensor_tensor(VR[:], VR[:], tmpV[:], ADD)
            vr2t = VR[:].rearrange("p (m d) -> p m d", m=16)

            # ---- Q projection: QT_all[d, h, b, a] ----
            QT = big.tile([128, 4 * S], BF16, tag="QT")
            qt4 = QT[:].rearrange("p (h b a) -> p h b a", h=4, b=32)
            for b in range(32):
                pq = psC.tile([128, 256], F32, tag="sc")
                nc.tensor.matmul(pq[:], wq_sb[:, b * 128:(b + 1) * 128],
                                 qT_sb[:], start=True, stop=True)
                nc.vector.tensor_scalar(
                    qt4[:, :, b, :], pq[:].rearrange("p (h a) -> p h a", h=4),
                    bq_sb[:, b:b + 1], None, ADD)

            # ---- attention per head (software-pipelined over jk) ----
            # Per (h, half) group: scores for tile jk are emitted two
            # iterations ahead of the av/dn matmuls that consume exp(jk),
            # so the PE never waits on the ScalarE exp.  PSUM budget:
            # av(2) + dn(2) + 2 in-flight sc tiles (4) = 8 banks.
            OHT = big.tile([128, 4 * S], BF16, tag="OHT")
            for h in range(4):
                for half in range(2):
                    base = h * S + half * 1024
                    av = psA.tile([128, 1024], F32, tag="av")
                    dn = psB.tile([128, 1024], F32, tag="dn")
                    prs = {}

                    def emit_sc(jk):
                        sc = psC.tile([128, 1024], F32, tag="sc")
                        for c in range(2):
                            nc.tensor.matmul(
                                sc[:, c * 512:(c + 1) * 512],
                                KT[:, jk * 128:(jk + 1) * 128],
                                QT[:, base + c * 512:base + (c + 1) * 512],
                                start=True, stop=True)
                        pr = pb.tile([128, 1024], BF16, tag="pr")
                        nc.scalar.activation(pr[:], sc[:], EXP, scale=SCALE)
                        prs[jk] = pr

                    def emit_avd(jk):
                        pr = prs.pop(jk)
                        for c in range(2):
                            cs = slice(c * 512, (c + 1) * 512)
                            nc.tensor.matmul(av[:, cs], vr2t[:, jk, :],
                                             pr[:, cs],
                                             start=(jk == 0),
                                             stop=(jk == 15),
                                             skip_group_check=True)
                            nc.tensor.matmul(dn[:, cs], onesf_sb[:],
                                             pr[:, cs],
                                             start=(jk == 0),
                                             stop=(jk == 15),
                                             skip_group_check=True)

                    emit_sc(0)
                    emit_sc(1)
                    for jk in range(16):
                        emit_avd(jk)
                        if jk + 2 < 16:
                            emit_sc(jk + 2)
                    rcp = rc.tile([128, 1024], F32, tag="rcp")
                    nc.vector.reciprocal_approx_fast(rcp[:], dn[:])
                    nc.vector.tensor_tensor(OHT[:, base:base + 1024],
                                            av[:], rcp[:], MUL)

            # ---- output projection (accumulate 4 heads) ----
            out_sb = big.tile([128, S], F32, tag="osb")
            for jc in range(4):
                po = psC.tile([128, 512], F32, tag="sc")
                for h in range(4):
                    nc.tensor.matmul(po[:],
                                     wo_sb[:, h * 128:(h + 1) * 128],
                                     OHT[:, h * S + jc * 512:
                                         h * S + (jc + 1) * 512],
                                     start=(h == 0), stop=(h == 3),
                                     skip_group_check=True)
                nc.vector.tensor_copy(out_sb[:, jc * 512:(jc + 1) * 512],
                                      po[:])
            nc.sync.dma_start(out=out[:], in_=out_sb[:])

    nc.compile()
    return nc


def _get_nc():
    if "nc" not in _nc_cache:
        _nc_cache["nc"] = _build_nc()
    return _nc_cache["nc"]


def make_in_maps(query, keys, values, Wq, bq, Wk, bk, Wv, bv, Wo, bo):
    BF = ml_dtypes.bfloat16
    cosK, sinKe, cosVr, sinVe = _rope_tables()
    cosK = cosK.astype(BF)
    sinKe = sinKe.astype(BF)
    cosVr = cosVr.astype(BF)
    sinVe = sinVe.astype(BF)
    q2 = np.asarray(query, np.float32).reshape(S, D)
    k2 = np.asarray(keys, np.float32).reshape(S, D)
    v2 = np.asarray(values, np.float32).reshape(S, D)
    Wq_ = np.ascontiguousarray(np.asarray(Wq, np.float32)).astype(BF)
    Wk_ = np.ascontiguousarray(np.asarray(Wk, np.float32)).astype(BF)
    Wv_ = np.ascontiguousarray(np.asarray(Wv, np.float32)).astype(BF)
    Wo_ = np.asarray(Wo, np.float32)
    bq_ = np.asarray(bq, np.float32).reshape(32, 128).T.copy()   # [d, b]
    bk_ = np.asarray(bk, np.float32).reshape(8, 128).T.copy()
    bv_ = np.asarray(bv, np.float32).reshape(1, KVH * D).astype(BF)
    ones_r = np.ones((1, 128), BF)
    ones_f = np.ones((128, 128), BF)

    in_maps = []
    for c in range(NCORES):
        heads = [c + 8 * r for r in range(REP)]
        qrows = np.concatenate([q2[hh * 64:(hh + 1) * 64] for hh in heads])
        woc = np.concatenate([Wo_[hh * 128:(hh + 1) * 128] for hh in heads],
                             axis=1)  # [128, 4*128]
        in_maps.append({
            "qT": np.ascontiguousarray(qrows.T).astype(BF),
            "kT": np.ascontiguousarray(k2[c * 256:(c + 1) * 256].T).astype(BF),
            "vT": np.ascontiguousarray(v2[c * 256:(c + 1) * 256].T).astype(BF),
            "wq": Wq_, "wk": Wk_, "wv": Wv_,
            "wo": np.ascontiguousarray(woc).astype(BF),
            "bq": bq_, "bk": bk_, "bv": bv_,
            "cosK": cosK, "sinK": sinKe, "cosV": cosVr, "sinV": sinVe,
            "onesr": ones_r, "onesf": ones_f,
        })
    return in_maps


def kernel(query, keys, values, Wq, bq, Wk, bk, Wv, bv, Wo, bo):
    nc = _get_nc()
    in_maps = make_in_maps(query, keys, values, Wq, bq, Wk, bk, Wv, bv, Wo, bo)
    res = run_bass_kernel_spmd(nc, in_maps, list(range(NCORES)))
    return postprocess(res.results, bo)


def postprocess(results, bo):
    acc = np.zeros((S, D), np.float64)
    for c in range(NCORES):
        o = np.asarray(results[c]["out"], np.float32)  # [dout=128, jq=2048]
        acc += o.T
    final = np.empty((S, D), np.float32)
    final[PERM_Q] = acc.astype(np.float32)
    final += np.asarray(bo, np.float32)
    return final.reshape(B, S, D)


# revision 7
# speedup vs baseline: 1.5789x; 1.0071x over previous
"""GQA kernel for trn2, 8 NeuronCores.

Problem: B=1, S=2048, D=128, H=32, KVH=8, REP=4, rope(theta=1e4) on k AND v,
softmax(q@k^T/sqrt(128)) @ v, out @ Wo + bo.  The reference replicates torch
.view() semantics: (B,S,H*D) -> (B,H,S,D) is a FLAT reinterpretation, so
q-head h is rows [h*64,(h+1)*64) of the projection output reinterpreted as
(2048,128), and kv-head g is rows [g*256,(g+1)*256) of the k/v projections.

Sharding: core c owns kv-head g=c and q-heads {c, c+8, c+16, c+24}.
Device keeps everything in matmul-natural "storage order": q-position
j_q = b*64+a  <-> actual s' = 32a+b, kv-position j_k = b*256+a <-> t = 8a+b.
RoPE tables are host-permuted into storage order; host un-permutes rows of
the final output and sums partials over cores (Wo is a per-head row-block
contraction, so per-core partials add).

Dataflow per head: scores^T[jk,jq] = (KTr2 slice).T @ QT slice (bf16),
exp via ScalarE psum->sbuf (bf16 probs), AV = V_r2[jk].T @ probs accumulated
in psum over jk tiles, denominators via all-ones lhsT matmul on the same
probs, fast-reciprocal + normalize on DVE, per-head Wo matmuls accumulate
output tiles.  The jk loop is software-pipelined: scores for tile jk+2 are
issued before the av/dn matmuls of tile jk so the PE never waits on exp.
Inputs arrive as 3 packed bf16 blobs + 1 small f32 bias tensor to minimize
DMA-issue serialization on the Sync queue; output leaves in 512-col chunks
overlapped with the tail of compute.
"""

import sys

sys.path.insert(0, "/opt/trn_rl_repo")

import numpy as np
import ml_dtypes

import concourse.bass as bass
import concourse.mybir as mybir
import concourse.tile as tile
from concourse import bacc
from concourse.bass_utils import run_bass_kernel_spmd

F32 = mybir.dt.float32
F32R = mybir.dt.float32r
BF16 = mybir.dt.bfloat16

B, S, D = 1, 2048, 128
H, KVH, REP = 32, 8, 4
NCORES = 8
SCALE = 1.0 / np.sqrt(128.0)
ROPE_THETA = 10000.0

# storage-order <-> position permutations
_j = np.arange(S)
PERM_Q = 32 * (_j % 64) + _j // 64          # s' = PERM_Q[j_q]
PERM_K = 8 * (_j % 256) + _j // 256         # t  = PERM_K[j_k]

# blob1: kT(256) wk(1024) vT(256) wv(1024) bv(1024,row0) onesr(128,row0)
#        onesf(128)
B1_KT, B1_WK, B1_VT, B1_WV, B1_BV, B1_ONESR, B1_ONESF, B1_END = (
    0, 256, 1280, 1536, 2560, 3584, 3712, 3840)
# blob2: qT(256) wq(4096) wo(512)
B2_QT, B2_WQ, B2_WO, B2_END = 0, 256, 4352, 4864
# blob3: cosK sinK cosV sinV (2048 each)
B3_END = 8192

_nc_cache = {}


def _rope_tables():
    inv_freq = 1.0 / (ROPE_THETA ** (np.arange(0, D, 2, dtype=np.float64) / D))
    ang = np.arange(S, dtype=np.float64)[:, None] * inv_freq  # (S, 64)
    cos = np.cos(ang)  # (S, 64), same for d and d+64
    sin = np.sin(ang)

    # K-transposed layout [d, j]: value at (d, j) uses t = PERM_K[j]
    cosK = np.empty((D, S), np.float32)
    sinKe = np.empty((D, S), np.float32)
    t = PERM_K
    cosK[:64, :] = cos[t, :].T
    cosK[64:, :] = cos[t, :].T
    sinKe[:64, :] = -sin[t, :].T   # rot[d<64] = -x[d+64]
    sinKe[64:, :] = sin[t, :].T    # rot[d>=64] = +x[d-64]

    # V row layout [p, m*128+d]: row j = m*128+p, t = PERM_K[j]
    cosVr = np.empty((128, S), np.float32)
    sinVe = np.empty((128, S), np.float32)
    for m in range(16):
        tj = PERM_K[m * 128 + np.arange(128)]
        c = cos[tj, :]  # (128, 64)
        s_ = sin[tj, :]
        cosVr[:, m * 128:m * 128 + 64] = c
        cosVr[:, m * 128 + 64:m * 128 + 128] = c
        sinVe[:, m * 128:m * 128 + 64] = -s_
        sinVe[:, m * 128 + 64:m * 128 + 128] = s_
    return cosK, sinKe, cosVr, sinVe


def _build_nc():
    nc = bacc.Bacc(None)
    dp = nc.declare_dram_parameter
    blob1 = dp("blob1", [128, B1_END], BF16, isOutput=False)
    blob2 = dp("blob2", [128, B2_END], BF16, isOutput=False)
    blob3 = dp("blob3", [128, B3_END], BF16, isOutput=False)
    bqk = dp("bqk", [128, 40], F32, isOutput=False)
    out = dp("out", [128, S], F32, isOutput=True)

    ADD = mybir.AluOpType.add
    MUL = mybir.AluOpType.mult
    EXP = mybir.ActivationFunctionType.Exp

    with tile.TileContext(nc) as tc:
        with tc.tile_pool(name="cst", bufs=1) as cst, \
             tc.tile_pool(name="big", bufs=1) as big, \
             tc.tile_pool(name="pb", bufs=4) as pb, \
             tc.tile_pool(name="rc", bufs=2) as rc, \
             tc.tile_pool(name="psA", bufs=1, space="PSUM") as psA, \
             tc.tile_pool(name="psB", bufs=1, space="PSUM") as psB, \
             tc.tile_pool(name="psC", bufs=2, space="PSUM") as psC:
            # ---- load inputs: 3 bf16 blobs + biases, in need order ----
            b1 = cst.tile([128, B1_END], BF16, tag="b1")
            b2 = cst.tile([128, B2_END], BF16, tag="b2")
            b3 = cst.tile([128, B3_END], BF16, tag="b3")
            bqk_sb = cst.tile([128, 40], F32, tag="bqk")
            nc.sync.dma_start(out=b1[:], in_=blob1[:])
            nc.sync.dma_start(out=bqk_sb[:], in_=bqk[:])
            nc.sync.dma_start(out=b2[:], in_=blob2[:])
            nc.sync.dma_start(out=b3[:], in_=blob3[:])
            kT_sb = b1[:, B1_KT:B1_KT + 256]
            wk_sb = b1[:, B1_WK:B1_WK + 1024]
            vT_sb = b1[:, B1_VT:B1_VT + 256]
            wv_sb = b1[:, B1_WV:B1_WV + 1024]
            bv_sb = b1[0:1, B1_BV:B1_BV + 1024]
            onesr_sb = b1[0:1, B1_ONESR:B1_ONESR + 128]
            onesf_sb = b1[:, B1_ONESF:B1_ONESF + 128]
            qT_sb = b2[:, B2_QT:B2_QT + 256]
            wq_sb = b2[:, B2_WQ:B2_WQ + 4096]
            wo_sb = b2[:, B2_WO:B2_WO + 512]
            cosK_sb = b3[:, 0:2048]
            sinK_sb = b3[:, 2048:4096]
            cosV_sb = b3[:, 4096:6144]
            sinV_sb = b3[:, 6144:8192]
            bq_sb = bqk_sb[:, 0:32]
            bk_sb = bqk_sb[:, 32:40]

            # ---- K projection + rope: KTr2[d, jk] ----
            KT = big.tile([128, S], BF16, tag="KT")
            for b in range(8):
                pk = psC.tile([128, 256], F32, tag="sc")
                nc.tensor.matmul(pk[:], wk_sb[:, b * 128:(b + 1) * 128],
                                 kT_sb, start=True, stop=True)
                nc.vector.tensor_scalar(KT[:, b * 256:(b + 1) * 256], pk[:],
                                        bk_sb[:, b:b + 1], None, ADD)
            tmpK = big.tile([128, S], BF16, tag="tmpK")
            nc.vector.tensor_copy(tmpK[0:64, :], KT[64:128, :])
            nc.vector.tensor_copy(tmpK[64:128, :], KT[0:64, :])
            nc.vector.tensor_tensor(tmpK[:], tmpK[:], sinK_sb, MUL)
            nc.vector.tensor_tensor(KT[:], KT[:], cosK_sb, MUL)
            nc.vector.tensor_tensor(KT[:], KT[:], tmpK[:], ADD)

            # ---- V projection + rope in row layout: V_r2[p, m, d] ----
            VR = big.tile([128, S], BF16, tag="VR")
            vr4 = VR[:].rearrange("p (b two d) -> p b two d", b=8, two=2)
            for bg in range(2):
                for ah in range(2):
                    pv = psC.tile([128, 512], F32, tag="sc")
                    nc.tensor.matmul(pv[:], onesr_sb,
                                     bv_sb[:, bg * 512:(bg + 1) * 512],
                                     start=True, stop=False)
                    nc.tensor.matmul(pv[:], vT_sb[:, ah * 128:(ah + 1) * 128],
                                     wv_sb[:, bg * 512:(bg + 1) * 512],
                                     start=False, stop=True,
                                     skip_group_check=True)
                    nc.vector.tensor_copy(
                        vr4[:, 4 * bg:4 * bg + 4, ah, :],
                        pv[:].rearrange("p (b d) -> p b d", b=4))
            tmpV = big.tile([128, S], BF16, tag="tmpV")
            vr3 = VR[:].rearrange("p (m h d) -> p m h d", m=16, h=2)
            tv3 = tmpV[:].rearrange("p (m h d) -> p m h d", m=16, h=2)
            sv3 = sinV_sb.rearrange("p (m h d) -> p m h d", m=16, h=2)
            nc.vector.tensor_tensor(tv3[:, :, 0, :], vr3[:, :, 1, :],
                                    sv3[:, :, 0, :], MUL)
            nc.vector.tensor_tensor(tv3[:, :, 1, :], vr3[:, :, 0, :],
                                    sv3[:, :, 1, :], MUL)
            nc.vector.tensor_tensor(VR[:], VR[:], cosV_sb, MUL)
            nc.vector.tensor_tensor(VR[:], VR[:], tmpV[:], ADD)
            vr2t = VR[:].rearrange("p (m d) -> p m d", m=16)

            # ---- Q projection: QT_all[d, h, b, a]; bias-add on ScalarE so
            # the DVE (busy with rope) is off the projection critical path ----
            QT = big.tile([128, 4 * S], BF16, tag="QT")
            qt4 = QT[:].rearrange("p (h b a) -> p h b a", h=4, b=32)
            for b in range(32):
                pq = psC.tile([128, 256], F32, tag="sc")
                nc.tensor.matmul(pq[:], wq_sb[:, b * 128:(b + 1) * 128],
                                 qT_sb, start=True, stop=True)
                nc.scalar.add(qt4[:, :, b, :],
                              pq[:].rearrange("p (h a) -> p h a", h=4),
                              bq_sb[:, b:b + 1])

            # ---- attention per head (software-pipelined over jk) ----
            # Per (h, half) group: scores for tile jk are emitted two
            # iterations ahead of the av/dn matmuls that consume exp(jk),
            # so the PE never waits on the ScalarE exp.  PSUM budget:
            # av(2) + dn(2) + 2 in-flight sc tiles (4) = 8 banks.
            OHT = big.tile([128, 4 * S], BF16, tag="OHT")
            out_sb = big.tile([128, S], F32, tag="osb")

            def emit_group(h, half):
                base = h * S + half * 1024
                av = psA.tile([128, 1024], F32, tag="av")
                dn = psB.tile([128, 1024], F32, tag="dn")
                prs = {}

                def emit_sc(jk):
                    sc = psC.tile([128, 1024], F32, tag="sc")
                    for c in range(2):
                        nc.tensor.matmul(
                            sc[:, c * 512:(c + 1) * 512],
                            KT[:, jk * 128:(jk + 1) * 128],
                            QT[:, base + c * 512:base + (c + 1) * 512],
                            start=True, stop=True)
                    pr = pb.tile([128, 1024], BF16, tag="pr")
                    nc.scalar.activation(pr[:], sc[:], EXP, scale=SCALE)
                    prs[jk] = pr

                def emit_avd(jk):
                    pr = prs.pop(jk)
                    for c in range(2):
                        cs = slice(c * 512, (c + 1) * 512)
                        nc.tensor.matmul(av[:, cs], vr2t[:, jk, :],
                                         pr[:, cs],
                                         start=(jk == 0), stop=(jk == 15),
                                         skip_group_check=True)
                        nc.tensor.matmul(dn[:, cs], onesf_sb,
                                         pr[:, cs],
                                         start=(jk == 0), stop=(jk == 15),
                                         skip_group_check=True)

                emit_sc(0)
                emit_sc(1)
                for jk in range(16):
                    emit_avd(jk)
                    if jk + 2 < 16:
                        emit_sc(jk + 2)
                rcp = rc.tile([128, 1024], F32, tag="rcp")
                nc.vector.reciprocal_approx_fast(rcp[:], dn[:])
                nc.vector.tensor_tensor(OHT[:, base:base + 1024],
                                        av[:], rcp[:], MUL)

            def emit_wo(jc):
                po = psC.tile([128, 512], F32, tag="sc")
                for h in range(4):
                    nc.tensor.matmul(po[:],
                                     wo_sb[:, h * 128:(h + 1) * 128],
                                     OHT[:, h * S + jc * 512:
                                         h * S + (jc + 1) * 512],
                                     start=(h == 0), stop=(h == 3),
                                     skip_group_check=True)
                nc.vector.tensor_copy(out_sb[:, jc * 512:(jc + 1) * 512],
                                      po[:])
                nc.sync.dma_start(out=out[:, jc * 512:(jc + 1) * 512],
                                  in_=out_sb[:, jc * 512:(jc + 1) * 512])

            # half-outer group order lets the Wo projection for the first
            # 1024 columns overlap the second half's attention groups.
            for half in range(2):
                for h in range(4):
                    emit_group(h, half)
                emit_wo(2 * half)
                emit_wo(2 * half + 1)

    nc.compile()
    return nc


def _get_nc():
    if "nc" not in _nc_cache:
        _nc_cache["nc"] = _build_nc()
    return _nc_cache["nc"]


def make_in_maps(query, keys, values, Wq, bq, Wk, bk, Wv, bv, Wo, bo):
    BF = ml_dtypes.bfloat16
    cosK, sinKe, cosVr, sinVe = _rope_tables()
    q2 = np.asarray(query, np.float32).reshape(S, D)
    k2 = np.asarray(keys, np.float32).reshape(S, D)
    v2 = np.asarray(values, np.float32).reshape(S, D)
    Wq_ = np.ascontiguousarray(np.asarray(Wq, np.float32))
    Wk_ = np.ascontiguousarray(np.asarray(Wk, np.float32))
    Wv_ = np.ascontiguousarray(np.asarray(Wv, np.float32))
    Wo_ = np.asarray(Wo, np.float32)
    bq_ = np.asarray(bq, np.float32).reshape(32, 128).T.copy()   # [d, b]
    bk_ = np.asarray(bk, np.float32).reshape(8, 128).T.copy()
    bv_ = np.asarray(bv, np.float32).reshape(1, KVH * D)

    bqk = np.zeros((128, 40), np.float32)
    bqk[:, 0:32] = bq_
    bqk[:, 32:40] = bk_

    blob3 = np.empty((128, B3_END), BF)
    blob3[:, 0:2048] = cosK.astype(BF)
    blob3[:, 2048:4096] = sinKe.astype(BF)
    blob3[:, 4096:6144] = cosVr.astype(BF)
    blob3[:, 6144:8192] = sinVe.astype(BF)

    in_maps = []
    for c in range(NCORES):
        heads = [c + 8 * r for r in range(REP)]
        qrows = np.concatenate([q2[hh * 64:(hh + 1) * 64] for hh in heads])
        woc = np.concatenate([Wo_[hh * 128:(hh + 1) * 128] for hh in heads],
                             axis=1)  # [128, 4*128]
        blob1 = np.zeros((128, B1_END), BF)
        blob1[:, B1_KT:B1_KT + 256] = k2[c * 256:(c + 1) * 256].T.astype(BF)
        blob1[:, B1_WK:B1_WK + 1024] = Wk_.astype(BF)
        blob1[:, B1_VT:B1_VT + 256] = v2[c * 256:(c + 1) * 256].T.astype(BF)
        blob1[:, B1_WV:B1_WV + 1024] = Wv_.astype(BF)
        blob1[0, B1_BV:B1_BV + 1024] = bv_[0].astype(BF)
        blob1[0, B1_ONESR:B1_ONESR + 128] = np.ones(128, BF)
        blob1[:, B1_ONESF:B1_ONESF + 128] = np.ones((128, 128), BF)
        blob2 = np.zeros((128, B2_END), BF)
        blob2[:, B2_QT:B2_QT + 256] = qrows.T.astype(BF)
        blob2[:, B2_WQ:B2_WQ + 4096] = Wq_.astype(BF)
        blob2[:, B2_WO:B2_WO + 512] = woc.astype(BF)
        in_maps.append({
            "blob1": blob1, "blob2": blob2, "blob3": blob3, "bqk": bqk,
        })
    return in_maps


def kernel(query, keys, values, Wq, bq, Wk, bk, Wv, bv, Wo, bo):
    nc = _get_nc()
    in_maps = make_in_maps(query, keys, values, Wq, bq, Wk, bk, Wv, bv, Wo, bo)
    res = run_bass_kernel_spmd(nc, in_maps, list(range(NCORES)))
    return postprocess(res.results, bo)


def postprocess(results, bo):
    acc = np.zeros((S, D), np.float64)
    for c in range(NCORES):
        o = np.asarray(results[c]["out"], np.float32)  # [dout=128, jq=2048]
        acc += o.T
    final = np.empty((S, D), np.float32)
    final[PERM_Q] = acc.astype(np.float32)
    final += np.asarray(bo, np.float32)
    return final.reshape(B, S, D)


# revision 13
# speedup vs baseline: 1.7628x; 1.1165x over previous
"""GQA kernel for trn2, 8 NeuronCores.

Problem: B=1, S=2048, D=128, H=32, KVH=8, REP=4, rope(theta=1e4) on k AND v,
softmax(q@k^T/sqrt(128)) @ v, out @ Wo + bo.  The reference replicates torch
.view() semantics: (B,S,H*D) -> (B,H,S,D) is a FLAT reinterpretation, so
q-head h is rows [h*64,(h+1)*64) of the projection output reinterpreted as
(2048,128), and kv-head g is rows [g*256,(g+1)*256) of the k/v projections.

Sharding: core c owns kv-head g=c and q-heads {c, c+8, c+16, c+24}.
Device keeps everything in matmul-natural "storage order": q-position
j_q = b*64+a  <-> actual s' = 32a+b, kv-position j_k = b*256+a <-> t = 8a+b.
RoPE tables are host-permuted into storage order; host un-permutes rows of
the final output and sums partials over cores (Wo is a per-head row-block
contraction, so per-core partials add).

Dataflow per head: scores^T[jk,jq] = (KTr2 slice).T @ QT slice (bf16),
exp via ScalarE psum->sbuf producing fp8e4 probs, AV and the all-ones
denominator matmuls run as fp8 DoubleRow over PAIRS of jk tiles (two
128-deep contractions per pass), fast-reciprocal + normalize on DVE,
per-head Wo matmuls accumulate output tiles.  The jk loop is
software-pipelined: scores are issued a pair ahead of the av/dn matmuls so
the PE never waits on exp.  Inputs arrive as packed bf16 blobs + 1 small
f32 bias tensor to minimize DMA-issue serialization; output leaves in
512-col chunks overlapped with the tail of compute.
"""

import sys

sys.path.insert(0, "/opt/trn_rl_repo")

import numpy as np
import ml_dtypes

import concourse.bass as bass
import concourse.mybir as mybir
import concourse.tile as tile
from concourse import bacc
from concourse.bass_utils import run_bass_kernel_spmd

F32 = mybir.dt.float32
BF16 = mybir.dt.bfloat16
FP8 = mybir.dt.float8e4
DR = mybir.MatmulPerfMode.DoubleRow

B, S, D = 1, 2048, 128
H, KVH, REP = 32, 8, 4
NCORES = 8
SCALE = 1.0 / np.sqrt(128.0)
ROPE_THETA = 10000.0

# storage-order <-> position permutations
_j = np.arange(S)
PERM_Q = 32 * (_j % 64) + _j // 64          # s' = PERM_Q[j_q]
PERM_K = 8 * (_j % 256) + _j // 256         # t  = PERM_K[j_k]

# blob1: kT(256) wk(1024) vT(256) wv(1024) bv(1024,row0) onesr(128,row0)
B1_KT, B1_WK, B1_VT, B1_WV, B1_BV, B1_ONESR, B1_END = (
    0, 256, 1280, 1536, 2560, 3584, 3712)
# blobK: cosK sinK;  blobV: cosV sinV
BK_END = 4096
BV_END = 4096
# blob2: qT(256) wq(4096) wo(512)
B2_QT, B2_WQ, B2_WO, B2_END = 0, 256, 4352, 4864

_nc_cache = {}


def _rope_tables():
    inv_freq = 1.0 / (ROPE_THETA ** (np.arange(0, D, 2, dtype=np.float64) / D))
    ang = np.arange(S, dtype=np.float64)[:, None] * inv_freq  # (S, 64)
    cos = np.cos(ang)  # (S, 64), same for d and d+64
    sin = np.sin(ang)

    # K-transposed layout [d, j]: value at (d, j) uses t = PERM_K[j]
    cosK = np.empty((D, S), np.float32)
    sinKe = np.empty((D, S), np.float32)
    t = PERM_K
    cosK[:64, :] = cos[t, :].T
    cosK[64:, :] = cos[t, :].T
    sinKe[:64, :] = -sin[t, :].T   # rot[d<64] = -x[d+64]
    sinKe[64:, :] = sin[t, :].T    # rot[d>=64] = +x[d-64]

    # V row layout [p, m*128+d]: row j = m*128+p, t = PERM_K[j]
    cosVr = np.empty((128, S), np.float32)
    sinVe = np.empty((128, S), np.float32)
    for m in range(16):
        tj = PERM_K[m * 128 + np.arange(128)]
        c = cos[tj, :]  # (128, 64)
        s_ = sin[tj, :]
        cosVr[:, m * 128:m * 128 + 64] = c
        cosVr[:, m * 128 + 64:m * 128 + 128] = c
        sinVe[:, m * 128:m * 128 + 64] = -s_
        sinVe[:, m * 128 + 64:m * 128 + 128] = s_
    return cosK, sinKe, cosVr, sinVe


def _build_nc():
    nc = bacc.Bacc(None)
    dp = nc.declare_dram_parameter
    blob1 = dp("blob1", [128, B1_END], BF16, isOutput=False)
    blobK = dp("blobK", [128, BK_END], BF16, isOutput=False)
    blob2 = dp("blob2", [128, B2_END], BF16, isOutput=False)
    blobV = dp("blobV", [128, BV_END], BF16, isOutput=False)
    bqk = dp("bqk", [128, 40], F32, isOutput=False)
    out = dp("out", [128, S], F32, isOutput=True)

    ADD = mybir.AluOpType.add
    MUL = mybir.AluOpType.mult
    EXP = mybir.ActivationFunctionType.Exp

    with tile.TileContext(nc) as tc:
        with tc.tile_pool(name="cst", bufs=1) as cst, \
             tc.tile_pool(name="big", bufs=1) as big, \
             tc.tile_pool(name="pb", bufs=4) as pb, \
             tc.tile_pool(name="rc", bufs=2) as rc, \
             tc.tile_pool(name="dna", bufs=2) as dna, \
             tc.tile_pool(name="psA", bufs=1, space="PSUM") as psA, \
             tc.tile_pool(name="psB", bufs=1, space="PSUM") as psB, \
             tc.tile_pool(name="psC", bufs=2, space="PSUM") as psC:
            # ---- load inputs: packed bf16 blobs + biases, in need order ----
            b1 = cst.tile([128, B1_END], BF16, tag="b1")
            bk_t = cst.tile([128, BK_END], BF16, tag="bkt")
            b2 = cst.tile([128, B2_END], BF16, tag="b2")
            bv_t = cst.tile([128, BV_END], BF16, tag="bvt")
            bqk_sb = cst.tile([128, 40], F32, tag="bqk")
            nc.sync.dma_start(out=b1[:], in_=blob1[:])
            nc.sync.dma_start(out=bqk_sb[:], in_=bqk[:])
            nc.sync.dma_start(out=bk_t[:], in_=blobK[:])
            nc.sync.dma_start(out=b2[:], in_=blob2[:])
            nc.sync.dma_start(out=bv_t[:], in_=blobV[:])
            kT_sb = b1[:, B1_KT:B1_KT + 256]
            wk_sb = b1[:, B1_WK:B1_WK + 1024]
            vT_sb = b1[:, B1_VT:B1_VT + 256]
            wv_sb = b1[:, B1_WV:B1_WV + 1024]
            bv_sb = b1[0:1, B1_BV:B1_BV + 1024]
            onesr_sb = b1[0:1, B1_ONESR:B1_ONESR + 128]
            cosK_sb = bk_t[:, 0:2048]
            sinK_sb = bk_t[:, 2048:4096]
            qT_sb = b2[:, B2_QT:B2_QT + 256]
            wq_sb = b2[:, B2_WQ:B2_WQ + 4096]
            wo_sb = b2[:, B2_WO:B2_WO + 512]
            cosV_sb = bv_t[:, 0:2048]
            sinV_sb = bv_t[:, 2048:4096]
            bq_sb = bqk_sb[:, 0:32]
            bk_sb = bqk_sb[:, 32:40]

            # bf16 all-ones for the denominator broadcast matmul
            onesp = big.tile([128, 128], BF16, tag="onesp")
            nc.gpsimd.memset(onesp[:], 1.0)

            # ---- K projection + rope: KTr2[d, jk] ----
            KT = big.tile([128, S], BF16, tag="KT")
            for b in range(8):
                pk = psC.tile([128, 256], F32, tag="sc")
                nc.tensor.matmul(pk[:], wk_sb[:, b * 128:(b + 1) * 128],
                                 kT_sb, start=True, stop=True)
                if b % 2 == 0:
                    nc.vector.tensor_scalar(KT[:, b * 256:(b + 1) * 256],
                                            pk[:], bk_sb[:, b:b + 1],
                                            None, ADD)
                else:
                    nc.scalar.add(KT[:, b * 256:(b + 1) * 256], pk[:],
                                  bk_sb[:, b:b + 1])
            tmpK = big.tile([128, S], BF16, tag="tmpK")
            nc.vector.tensor_copy(tmpK[0:64, :], KT[64:128, :])
            nc.vector.tensor_copy(tmpK[64:128, :], KT[0:64, :])
            nc.vector.tensor_tensor(tmpK[:], tmpK[:], sinK_sb, MUL)
            nc.vector.tensor_tensor(KT[:], KT[:], cosK_sb, MUL)
            nc.vector.tensor_tensor(KT[:], KT[:], tmpK[:], ADD)

            # ---- V projection + rope in row layout: V_r2[p, m, d] ----
            VR = big.tile([128, S], BF16, tag="VR")
            vr4 = VR[:].rearrange("p (b two d) -> p b two d", b=8, two=2)
            for bg in range(2):
                for ah in range(2):
                    pv = psC.tile([128, 512], F32, tag="sc")
                    nc.tensor.matmul(pv[:], onesr_sb,
                                     bv_sb[:, bg * 512:(bg + 1) * 512],
                                     start=True, stop=False)
                    nc.tensor.matmul(pv[:], vT_sb[:, ah * 128:(ah + 1) * 128],
                                     wv_sb[:, bg * 512:(bg + 1) * 512],
                                     start=False, stop=True,
                                     skip_group_check=True)
                    if ah == 0:
                        nc.vector.tensor_copy(
                            vr4[:, 4 * bg:4 * bg + 4, ah, :],
                            pv[:].rearrange("p (b d) -> p b d", b=4))
                    else:
                        nc.scalar.copy(
                            vr4[:, 4 * bg:4 * bg + 4, ah, :],
                            pv[:].rearrange("p (b d) -> p b d", b=4))
            tmpV = big.tile([128, S], BF16, tag="tmpV")
            vr3 = VR[:].rearrange("p (m h d) -> p m h d", m=16, h=2)
            tv3 = tmpV[:].rearrange("p (m h d) -> p m h d", m=16, h=2)
            sv3 = sinV_sb.rearrange("p (m h d) -> p m h d", m=16, h=2)
            nc.vector.tensor_tensor(tv3[:, :, 0, :], vr3[:, :, 1, :],
                                    sv3[:, :, 0, :], MUL)
            nc.vector.tensor_tensor(tv3[:, :, 1, :], vr3[:, :, 0, :],
                                    sv3[:, :, 1, :], MUL)
            nc.vector.tensor_tensor(VR[:], VR[:], cosV_sb, MUL)
            nc.vector.tensor_tensor(VR[:], VR[:], tmpV[:], ADD)
            vr2t = VR[:].rearrange("p (m d) -> p m d", m=16)

            # ---- Q projection: QT_all[d, h, b, a]; alternate the psum->sbuf
            # bias-add between DVE and ScalarE so neither gates the PE ----
            QT = big.tile([128, 4 * S], BF16, tag="QT")
            qt4 = QT[:].rearrange("p (h b a) -> p h b a", h=4, b=32)
            for b in range(32):
                pq = psC.tile([128, 256], F32, tag="sc")
                nc.tensor.matmul(pq[:], wq_sb[:, b * 128:(b + 1) * 128],
                                 qT_sb, start=True, stop=True)
                if b % 2 == 0:
                    nc.vector.tensor_scalar(
                        qt4[:, :, b, :],
                        pq[:].rearrange("p (h a) -> p h a", h=4),
                        bq_sb[:, b:b + 1], None, ADD)
                else:
                    nc.scalar.add(qt4[:, :, b, :],
                                  pq[:].rearrange("p (h a) -> p h a", h=4),
                                  bq_sb[:, b:b + 1])

            # ---- attention per head (software-pipelined over jk pairs) ----
            # PSUM budget: av(2) + dn(2) + 2 in-flight sc tiles (4) = 8 banks.
            OHT = big.tile([128, 4 * S], BF16, tag="OHT")
            out_sb = big.tile([128, S], F32, tag="osb")

            def emit_group(h, half):
                base = h * S + half * 1024
                av = psA.tile([128, 1024], F32, tag="av")
                # bf16 elementwise accumulator over jk tiles; the cross-
                # partition denominator sum happens in one ones-matmul pair
                # at group end (saves 30 PE matmuls per group vs per-tile
                # ones-matmuls).
                dnacc = dna.tile([128, 1024], BF16, tag="dnacc",
                                 name="dnacc")
                prs = {}

                def emit_sc(jk):
                    sc = psC.tile([128, 1024], F32, tag="sc")
                    for c in range(2):
                        nc.tensor.matmul(
                            sc[:, c * 512:(c + 1) * 512],
                            KT[:, jk * 128:(jk + 1) * 128],
                            QT[:, base + c * 512:base + (c + 1) * 512],
                            start=True, stop=True)
                    pr = pb.tile([128, 1024], BF16, tag="pr", name="pr")
                    nc.scalar.activation(pr[:], sc[:], EXP, scale=SCALE)
                    prs[jk] = pr

                def emit_avd(jk):
                    pr = prs.pop(jk)
                    for c in range(2):
                        cs = slice(c * 512, (c + 1) * 512)
                        nc.tensor.matmul(av[:, cs], vr2t[:, jk, :],
                                         pr[:, cs],
                                         start=(jk == 0), stop=(jk == 15),
                                         skip_group_check=True)
                    if jk == 0:
                        nc.vector.tensor_copy(dnacc[:], pr[:])
                    else:
                        nc.vector.tensor_tensor(dnacc[:], dnacc[:], pr[:],
                                                ADD)

                emit_sc(0)
                emit_sc(1)
                for jk in range(16):
                    emit_avd(jk)
                    if jk + 2 < 16:
                        emit_sc(jk + 2)
                dnbc = psB.tile([128, 1024], F32, tag="dnbc", name="dnbc")
                for c in range(2):
                    cs = slice(c * 512, (c + 1) * 512)
                    nc.tensor.matmul(dnbc[:, cs], onesp[:], dnacc[:, cs],
                                     start=True, stop=True)
                rcp = rc.tile([128, 1024], F32, tag="rcp")
                nc.vector.reciprocal_approx_fast(rcp[:], dnbc[:])
                nc.vector.tensor_tensor(OHT[:, base:base + 1024],
                                        av[:], rcp[:], MUL)

            def emit_wo(jc):
                po = psC.tile([128, 512], F32, tag="sc")
                for h in range(4):
                    nc.tensor.matmul(po[:],
                                     wo_sb[:, h * 128:(h + 1) * 128],
                                     OHT[:, h * S + jc * 512:
                                         h * S + (jc + 1) * 512],
                                     start=(h == 0), stop=(h == 3),
                                     skip_group_check=True)
                nc.vector.tensor_copy(out_sb[:, jc * 512:(jc + 1) * 512],
                                      po[:])
                nc.sync.dma_start(out=out[:, jc * 512:(jc + 1) * 512],
                                  in_=out_sb[:, jc * 512:(jc + 1) * 512])

            # half-outer group order lets the Wo projection for the first
            # 1024 columns overlap the second half's attention groups.
            for half in range(2):
                for h in range(4):
                    emit_group(h, half)
                emit_wo(2 * half)
                emit_wo(2 * half + 1)

    nc.compile()
    return nc


def _get_nc():
    if "nc" not in _nc_cache:
        _nc_cache["nc"] = _build_nc()
    return _nc_cache["nc"]


def make_in_maps(query, keys, values, Wq, bq, Wk, bk, Wv, bv, Wo, bo):
    BF = ml_dtypes.bfloat16
    cosK, sinKe, cosVr, sinVe = _rope_tables()
    q2 = np.asarray(query, np.float32).reshape(S, D)
    k2 = np.asarray(keys, np.float32).reshape(S, D)
    v2 = np.asarray(values, np.float32).reshape(S, D)
    Wq_ = np.ascontiguousarray(np.asarray(Wq, np.float32))
    Wk_ = np.ascontiguousarray(np.asarray(Wk, np.float32))
    Wv_ = np.ascontiguousarray(np.asarray(Wv, np.float32))
    Wo_ = np.asarray(Wo, np.float32)
    bq_ = np.asarray(bq, np.float32).reshape(32, 128).T.copy()   # [d, b]
    bk_ = np.asarray(bk, np.float32).reshape(8, 128).T.copy()
    bv_ = np.asarray(bv, np.float32).reshape(1, KVH * D)

    bqk = np.zeros((128, 40), np.float32)
    bqk[:, 0:32] = bq_
    bqk[:, 32:40] = bk_

    blobK = np.empty((128, BK_END), BF)
    blobK[:, 0:2048] = cosK.astype(BF)
    blobK[:, 2048:4096] = sinKe.astype(BF)
    blobV = np.empty((128, BV_END), BF)
    blobV[:, 0:2048] = cosVr.astype(BF)
    blobV[:, 2048:4096] = sinVe.astype(BF)

    in_maps = []
    for c in range(NCORES):
        heads = [c + 8 * r for r in range(REP)]
        qrows = np.concatenate([q2[hh * 64:(hh + 1) * 64] for hh in heads])
        woc = np.concatenate([Wo_[hh * 128:(hh + 1) * 128] for hh in heads],
                             axis=1)  # [128, 4*128]
        blob1 = np.zeros((128, B1_END), BF)
        blob1[:, B1_KT:B1_KT + 256] = k2[c * 256:(c + 1) * 256].T.astype(BF)
        blob1[:, B1_WK:B1_WK + 1024] = Wk_.astype(BF)
        blob1[:, B1_VT:B1_VT + 256] = v2[c * 256:(c + 1) * 256].T.astype(BF)
        blob1[:, B1_WV:B1_WV + 1024] = Wv_.astype(BF)
        blob1[0, B1_BV:B1_BV + 1024] = bv_[0].astype(BF)
        blob1[0, B1_ONESR:B1_ONESR + 128] = np.ones(128, BF)
        blob2 = np.zeros((128, B2_END), BF)
        blob2[:, B2_QT:B2_QT + 256] = qrows.T.astype(BF)
        blob2[:, B2_WQ:B2_WQ + 4096] = Wq_.astype(BF)
        blob2[:, B2_WO:B2_WO + 512] = woc.astype(BF)
        in_maps.append({
            "blob1": blob1, "blobK": blobK, "blob2": blob2, "blobV": blobV,
            "bqk": bqk,
        })
    return in_maps


def kernel(query, keys, values, Wq, bq, Wk, bk, Wv, bv, Wo, bo):
    nc = _get_nc()
    in_maps = make_in_maps(query, keys, values, Wq, bq, Wk, bk, Wv, bv, Wo, bo)
    res = run_bass_kernel_spmd(nc, in_maps, list(range(NCORES)))
    return postprocess(res.results, bo)


def postprocess(results, bo):
    acc = np.zeros((S, D), np.float64)
    for c in range(NCORES):
        o = np.asarray(results[c]["out"], np.float32)  # [dout=128, jq=2048]
        acc += o.T
    final = np.empty((S, D), np.float32)
    final[PERM_Q] = acc.astype(np.float32)
    final += np.asarray(bo, np.float32)
    return final.reshape(B, S, D)
